# revision 1
# baseline (speedup 1.0000x reference)
"""Trainium2 Bass kernel for a local-window self-attention block (v6: +PSUM slack).

Reference computation (fp32):
    qkv = val @ w_qkv + b_qkv                  # [B,S,3D]
    per (batch, window of 1024, head): local softmax attention
    out = LayerNorm(val + ctx @ w_out + b_out) * gamma + beta

Sharding: B=2 batches x 4 windows of W=1024 tokens = 8 fully independent
shards (attention windows are independent; projections and LN are
per-token), one per NeuronCore. No collectives.

Per-core layout strategy (T=1024 tokens, D=1024, H=8 heads, DH=128):
  - xT[d, t] built via PE transpose; all matmuls contract over the
    partition axis with fp32r (full-rate fp32 PE mode).
  - Q^T/K^T computed weights-stationary -> [3D? 2D, T] spilled to a DRAM
    scratch and streamed back per head; V computed tokens-stationary
    -> [T, D] kept resident (exactly the AV lhsT layout).
  - scores are computed TRANSPOSED: S^T[k, q] = K_h @ Q_h^T so softmax
    exp runs on ScalarE straight out of PSUM (scale=1/sqrt(DH) fused)
    and the denominator is a ones-column matmul (sum over partitions).
    max-subtraction is skipped: |scores| < ~6 for this data regime, and
    softmax is shift-invariant so the result is identical in fp32 range.
  - ctx^T = V_h^T @ E^T accumulated in PSUM, normalized by the
    reciprocal denominator broadcast across partitions (GpSimd).
  - out-proj runs ctx-stationary producing attn_out in [T, D] natural
    layout, where residual add + LayerNorm (bn_stats/bn_aggr) finish.
"""

import numpy as np

T = 1024  # tokens per shard (window)
D = 1024
H = 8
DH = 128
P = 128
NT = T // P  # 8 token tiles
ND = D // P  # 8 d tiles
NCORES = 8
SCALE = 1.0 / float(np.sqrt(DH))
EPS = 1e-5

_CACHE = {}


def _emit_body(nc, tc, pools, aps):
    _emit_phases(nc, tc, pools, aps, 5)


def _emit_phases(nc, tc, pools, aps, upto):
    import concourse.mybir as mybir

    f32 = mybir.dt.float32
    f32r = mybir.dt.float32r
    AF = mybir.ActivationFunctionType

    (x, w_qkv, b_qkv, w_out, b_out, ln_gamma, ln_beta, y, qkT_dram) = aps
    singles, bcast, xio, arena, vpool, qkth, epool, rbpool, wqk, wmov, rrow, mm, psd = (
        pools
    )

    # ---- constants ----
    from concourse.masks import make_identity

    ident = singles.tile([P, P], f32, tag="ident")
    make_identity(nc, ident[:])
    ones_f32 = singles.tile([P, 1], f32, tag="ones32")
    nc.vector.memset(ones_f32[:], 1.0)
    ones_col = singles.tile([P, 1], f32r, tag="ones")
    nc.vector.tensor_copy(ones_col[:], ones_f32[:])
    eps_t = singles.tile([P, 1], f32, tag="eps")
    nc.vector.memset(eps_t[:], EPS)
    bqk_all = singles.tile([P, 2 * ND], f32, tag="bqk")
    nc.sync.dma_start(
        out=bqk_all[:], in_=b_qkv[0 : 2 * D].rearrange("(m p) -> p m", p=P)
    )
    bv_bc = bcast.tile([P, D], f32, tag="bv")
    nc.sync.dma_start(out=bv_bc[:], in_=b_qkv[2 * D : 3 * D].partition_broadcast(P))
    bout_bc = bcast.tile([P, D], f32, tag="bout")
    nc.sync.dma_start(out=bout_bc[:], in_=b_out.partition_broadcast(P))
    gamma_bc = bcast.tile([P, D], f32, tag="gamma")
    nc.sync.dma_start(out=gamma_bc[:], in_=ln_gamma.partition_broadcast(P))
    beta_bc = bcast.tile([P, D], f32, tag="beta")
    nc.sync.dma_start(out=beta_bc[:], in_=ln_beta.partition_broadcast(P))

    # ---- phase A: load x and build xT via PE transpose ----
    xT = []
    for dd in range(ND):
        xT.append(arena.tile([P, T], f32r, tag="arena", name="arena_t"))
    for tt in range(NT):
        x_in = xio.tile([P, D], f32, tag="xio", name="xio_t")
        nc.sync.dma_start(out=x_in[:], in_=x[tt * P : (tt + 1) * P, :])
        for dd in range(ND):
            ps = mm.tile([P, P], f32, tag="mm", name="mmtr_t")
            nc.tensor.transpose(ps[:], x_in[:, dd * P : (dd + 1) * P], ident[:])
            nc.scalar.activation(
                out=xT[dd][:, tt * P : (tt + 1) * P],
                in_=ps[:],
                func=AF.Identity,
                bias=0.0,
                scale=1.0,
            )

    if upto < 2:
        return
    # ---- phase B1: Q^T / K^T projection -> DRAM scratch ----
    # qkT_dram[m*128:(m+1)*128, :] = (x @ w_qkv[:, m*128:(m+1)*128]).T + bias
    for m in [v for h_ in range(ND) for v in (h_, ND + h_)]:
        wq = wqk.tile([P, ND, P], f32r, tag="wqk", name="wqk_t")
        nc.sync.dma_start(
            out=wq[:],
            in_=w_qkv[:, m * P : (m + 1) * P]
            .rearrange("(kt kp) n -> kp kt n", kp=P)
            .bitcast(f32r),
        )
        for ch in range(2):
            ps = mm.tile([P, 512], f32, tag="mm", name="mm_t")
            for k in range(ND):
                nc.tensor.matmul(
                    ps[:],
                    wq[:, k, :],
                    xT[k][:, ch * 512 : (ch + 1) * 512],
                    start=(k == 0),
                    stop=(k == ND - 1),
                )
            stage = wmov.tile([P, 512], f32r, tag="wmov", name="wmov_t")
            nc.scalar.activation(
                out=stage[:],
                in_=ps[:],
                func=AF.Identity,
                bias=bqk_all[:, m : m + 1],
                scale=1.0,
            )
            nc.sync.dma_start(
                out=qkT_dram[m * P : (m + 1) * P, ch * 512 : (ch + 1) * 512],
                in_=stage[:],
            )

    if upto < 3:
        return
    # ---- phase B2: V projection (tokens-stationary) -> resident V [T, D] ----
    V = []
    for tt in range(NT):
        V.append(vpool.tile([P, D], f32r, tag="v", name="v_t"))
    for ch in range(2):
        wv = []
        for k in range(ND):
            wvk = wmov.tile([P, 512], f32r, tag="wmov", name="wmov_t")
            nc.sync.dma_start(
                out=wvk[:],
                in_=w_qkv[
                    k * P : (k + 1) * P, 2 * D + ch * 512 : 2 * D + (ch + 1) * 512
                ].bitcast(f32r),
            )
            wv.append(wvk)
        for tt in range(NT):
            ps = mm.tile([P, 512], f32, tag="mm", name="mm_t")
            for k in range(ND):
                nc.tensor.matmul(
                    ps[:],
                    xT[k][:, tt * P : (tt + 1) * P],
                    wv[k][:],
                    start=(k == 0),
                    stop=(k == ND - 1),
                )
            nc.vector.tensor_add(
                V[tt][:, ch * 512 : (ch + 1) * 512],
                ps[:],
                bv_bc[:, ch * 512 : (ch + 1) * 512],
            )

    if upto < 4:
        return
    # ---- phase C: per-head attention ----
    ctx = []
    for h in range(H):
        # stream Q^T, K^T for this head back from scratch
        qT = qkth.tile([P, T], f32r, tag="qkth", name="qkth_t")
        nc.sync.dma_start(out=qT[:], in_=qkT_dram[h * P : (h + 1) * P, :])
        kT = qkth.tile([P, T], f32r, tag="qkth", name="qkth_t")
        nc.sync.dma_start(out=kT[:], in_=qkT_dram[(ND + h) * P : (ND + h + 1) * P, :])

        # scores^T = K_h @ Q_h^T, exp fused with 1/sqrt(DH) scale
        E = [[None] * NT for _ in range(2)]
        for ch in range(2):
            for kt in range(NT):
                ps = mm.tile([P, 512], f32, tag="mm", name="mm_t")
                nc.tensor.matmul(
                    ps[:],
                    kT[:, kt * P : (kt + 1) * P],
                    qT[:, ch * 512 : (ch + 1) * 512],
                    start=True,
                    stop=True,
                )
                e = epool.tile([P, 512], f32r, tag="e", name="e_t")
                nc.scalar.activation(
                    out=e[:], in_=ps[:], func=AF.Exp, bias=0.0, scale=SCALE
                )
                E[ch][kt] = e

        # denominators: ones-column matmul sums E over keys (partitions)
        r_row = rrow.tile([1, T], f32, tag="rrow", name="rrow_t")
        for ch in range(2):
            psd_t = psd.tile([1, 512], f32, tag="psd", name="psd_t")
            for kt in range(NT):
                nc.tensor.matmul(
                    psd_t[:],
                    ones_col[:],
                    E[ch][kt][:],
                    start=(kt == 0),
                    stop=(kt == NT - 1),
                )
            nc.vector.reciprocal(r_row[:, ch * 512 : (ch + 1) * 512], psd_t[:])
        rb = rbpool.tile([P, T], f32, tag="rb", name="rb_t")
        nc.gpsimd.partition_broadcast(rb[:], r_row[:])

        # ctx^T = V_h^T @ E^T, then normalize by broadcast reciprocal
        c = arena.tile([P, T], f32r, tag="arena", name="arena_t")
        for ch in range(2):
            ps = mm.tile([P, 512], f32, tag="mm", name="mm_t")
            for kt in range(NT):
                nc.tensor.matmul(
                    ps[:],
                    V[kt][:, h * P : (h + 1) * P],
                    E[ch][kt][:],
                    start=(kt == 0),
                    stop=(kt == NT - 1),
                )
            nc.vector.tensor_mul(
                c[:, ch * 512 : (ch + 1) * 512],
                ps[:],
                rb[:, ch * 512 : (ch + 1) * 512],
            )
        ctx.append(c)

    if upto < 5:
        return
    # ---- phase D: out projection + residual + LayerNorm ----
    wout = []
    for k in range(ND):
        per_ch = []
        for ch in range(2):
            wo = epool.tile([P, 512], f32r, tag="e", name="wo_t")
            nc.sync.dma_start(
                out=wo[:],
                in_=w_out[k * P : (k + 1) * P, ch * 512 : (ch + 1) * 512].bitcast(f32r),
            )
            per_ch.append(wo)
        wout.append(per_ch)
    for tt in range(NT):
        x2 = xio.tile([P, D], f32, tag="xio", name="xio_t")
        nc.sync.dma_start(out=x2[:], in_=x[tt * P : (tt + 1) * P, :])
        res = xio.tile([P, D], f32, tag="xio", name="xio_t")
        for ch in range(2):
            ps = mm.tile([P, 512], f32, tag="mm", name="mm_t")
            for k in range(ND):
                nc.tensor.matmul(
                    ps[:],
                    ctx[k][:, tt * P : (tt + 1) * P],
                    wout[k][ch][:],
                    start=(k == 0),
                    stop=(k == ND - 1),
                )
            sl = slice(ch * 512, (ch + 1) * 512)
            nc.vector.tensor_add(res[:, sl], ps[:], bout_bc[:, sl])
            nc.vector.tensor_add(res[:, sl], res[:, sl], x2[:, sl])

        # LayerNorm over D (free axis; per-token stats are per-partition)
        stats = rrow.tile([P, 2, 6], f32, tag="stats", name="stats_t")
        mv = rrow.tile([P, 2], f32, tag="mv", name="mv_t")
        grouped = res[:].rearrange("p (g d) -> p g d", g=2)
        for g in range(2):
            nc.vector.bn_stats(out=stats[:, g, :], in_=grouped[:, g, :])
        nc.vector.bn_aggr(out=mv[:], in_=stats[:])
        rstd = rrow.tile([P, 1], f32, tag="rstd", name="rstd_t")
        nc.scalar.activation(
            out=rstd[:], in_=mv[:, 1:2], func=AF.Sqrt, bias=eps_t[:], scale=1.0
        )
        nc.vector.reciprocal(rstd[:], rstd[:])
        nc.vector.tensor_scalar(
            out=res[:],
            in0=res[:],
            scalar1=mv[:, 0:1],
            scalar2=rstd[:],
            op0=mybir.AluOpType.subtract,
            op1=mybir.AluOpType.mult,
        )
        nc.vector.tensor_mul(res[:], res[:], gamma_bc[:])
        nc.vector.tensor_add(res[:], res[:], beta_bc[:])
        nc.sync.dma_start(out=y[tt * P : (tt + 1) * P, :], in_=res[:])


def build(n_iters: int = 1):
    import concourse.mybir as mybir
    import concourse.tile as tile
    from concourse import bacc

    f32 = mybir.dt.float32

    nc = bacc.Bacc(
        "TRN2", target_bir_lowering=False, debug=False, num_devices=NCORES
    )
    x = nc.dram_tensor("x", [T, D], f32, kind="ExternalInput").ap()
    w_qkv = nc.dram_tensor("w_qkv", [D, 3 * D], f32, kind="ExternalInput").ap()
    b_qkv = nc.dram_tensor("b_qkv", [3 * D], f32, kind="ExternalInput").ap()
    w_out = nc.dram_tensor("w_out", [D, D], f32, kind="ExternalInput").ap()
    b_out = nc.dram_tensor("b_out", [D], f32, kind="ExternalInput").ap()
    ln_gamma = nc.dram_tensor("ln_gamma", [D], f32, kind="ExternalInput").ap()
    ln_beta = nc.dram_tensor("ln_beta", [D], f32, kind="ExternalInput").ap()
    y = nc.dram_tensor("y", [T, D], f32, kind="ExternalOutput").ap()
    qkT_dram = nc.dram_tensor("qkT_scratch", [2 * D, T], mybir.dt.float32r).ap()
    aps = (x, w_qkv, b_qkv, w_out, b_out, ln_gamma, ln_beta, y, qkT_dram)

    with tile.TileContext(nc) as tc:
        with (
            tc.tile_pool(name="singles", bufs=1) as singles,
            tc.tile_pool(name="bcast", bufs=1) as bcast,
            tc.tile_pool(name="xio", bufs=3) as xio,
            tc.tile_pool(name="arena", bufs=10) as arena,
            tc.tile_pool(name="vpool", bufs=8) as vpool,
            tc.tile_pool(name="qkth", bufs=4) as qkth,
            tc.tile_pool(name="epool", bufs=18) as epool,
            tc.tile_pool(name="rbpool", bufs=1) as rbpool,
            tc.tile_pool(name="wqk", bufs=2) as wqk,
            tc.tile_pool(name="wmov", bufs=10) as wmov,
            tc.tile_pool(name="rrow", bufs=2) as rrow,
            tc.tile_pool(name="mm", bufs=6, space="PSUM") as mm,
            tc.tile_pool(name="psd", bufs=2, space="PSUM") as psd,
        ):
            pools = (
                singles, bcast, xio, arena, vpool, qkth, epool, rbpool,
                wqk, wmov, rrow, mm, psd,
            )
            if n_iters == 1:
                _emit_body(nc, tc, pools, aps)
            else:
                with tc.For_i(0, n_iters, 1):
                    _emit_body(nc, tc, pools, aps)
    nc.compile()
    return nc


def _get_nc(n_iters: int = 1):
    key = n_iters
    if key not in _CACHE:
        _CACHE[key] = build(n_iters)
    return _CACHE[key]


def _shard_inputs(inputs):
    val = np.ascontiguousarray(inputs["val"], dtype=np.float32)
    B, S, Dm = val.shape
    shards = val.reshape(B * (S // T), T, Dm)
    shared = {
        "w_qkv": np.ascontiguousarray(inputs["w_qkv"], dtype=np.float32),
        "b_qkv": np.ascontiguousarray(inputs["b_qkv"], dtype=np.float32),
        "w_out": np.ascontiguousarray(inputs["w_out"], dtype=np.float32),
        "b_out": np.ascontiguousarray(inputs["b_out"], dtype=np.float32),
        "ln_gamma": np.ascontiguousarray(inputs["ln_gamma"], dtype=np.float32),
        "ln_beta": np.ascontiguousarray(inputs["ln_beta"], dtype=np.float32),
    }
    in_maps = []
    for i in range(NCORES):
        m = {"x": np.ascontiguousarray(shards[i])}
        m.update(shared)
        in_maps.append(m)
    return in_maps, (B, S, Dm)



def _setup_jax_cache():
    import os
    d = os.environ.get("JAX_COMPILATION_CACHE_DIR") or os.path.expanduser(
        "~/.cache/bass_kernel_jax_cache"
    )
    try:
        os.makedirs(d, exist_ok=True)
        import jax

        jax.config.update("jax_compilation_cache_dir", d)
        jax.config.update("jax_persistent_cache_min_compile_time_secs", 1.0)
    except Exception:
        pass


def run_on_cores(inputs, n_iters: int = 1):
    _setup_jax_cache()
    from concourse.bass_utils import run_bass_kernel_spmd

    nc = _get_nc(n_iters)
    in_maps, shape = _shard_inputs(inputs)
    res = run_bass_kernel_spmd(nc, in_maps, list(range(NCORES)))
    B, S, Dm = shape
    out = np.stack([res.results[i]["y"] for i in range(NCORES)], axis=0)
    return out.reshape(B, S, Dm)


def kernel(**inputs) -> np.ndarray:
    return run_on_cores(inputs, n_iters=1)

